# revision 51
# baseline (speedup 1.0000x reference)
"""Single-head causal attention (B=4, T=4096, D=512, H=128) on 8 TRN2 NeuronCores.

Sharding: data-parallel over batch (4 batches x 2 cores). The two cores of a
batch split the 32 query tiles zig-zag style so causal work is balanced
(each core gets one long-context and one short-context tile per pair).
One SPMD program serves both core "types": per-slot k-tile counts are padded
to a shared compile-time schedule, and the causal boundary is applied with
per-core 0/1 mask tiles supplied as input data (host precomputed, DMA'd).

Per-core device program (all matmuls fp16 inputs -> fp32 PSUM):
  K^T = (wk^T @ x^T), V = x @ wv + bv, Q^T = (wq'^T @ xq^T), wq' = wq/sqrt(H)
  per slot group (4 slots, k-outer): S^T[k,q] = K^T_tile.T @ Q^T block
  P = exp(S^T) (no max subtraction: scores are O(5), fp16 holds exp fine),
  boundary tiles multiplied by 0/1 masks, O[q, 0:129] += P^T.T @ [V | 1]
  out = O[:, :128] * (1 / O[:, 128]).
The ones-column of V carries the softmax denominator through the same PSUM
accumulation, so no running max/sum bookkeeping is needed.

Startup engineering (v2): strip-major DRAM layout gives 4KB-contiguous DMA
lines per partition; weights go out first on the gpsimd (SWDGE) queue while
x strips stream on the two HWDGE queues; a dummy exp preloads the ScalarE
activation table set; and a short chain of throwaway matmuls on scratch SBUF
warms the PE HAM clock gate (1.2 -> 2.4 GHz) before real work arrives.
"""

import numpy as np
import ml_dtypes

B, T, D, H = 4, 4096, 512, 128
P = 128          # partitions / tile edge
DO = D // P      # contraction chunks (4)
NT = T // P      # k tiles per batch (32)
NS = 16          # query-tile slots per core
TQ = NS * P      # queries per core (2048)
KC = [32 - 2 * s for s in range(NS)]   # k-tiles processed per slot (desc)
NSTRIP = T // 512          # 8 key strips
NQSTRIP = TQ // 512        # 4 query strips (one per slot group)
NWARM = 6                  # HAM warmup matmuls
CW = 3 * D                 # packed const blob: wq|wk|wv chunks (f16 cols)

_f16 = np.float16

_CACHE = {}


def _slot_qtile(core_type: int):
    """Global q-tile index handled by each slot, for core type 0/1."""
    out = []
    for s in range(NS):
        if s < 8:
            j = 31 - 2 * s - core_type          # long-context slots
        else:
            p = 15 - s
            j = 2 * p + core_type               # short-context slots
        out.append(j)
    return out


def _build_program():
    import concourse.tile as tile
    from concourse import bacc, mybir
    from concourse.bass import ts, ds

    f16 = mybir.dt.float16
    f32 = mybir.dt.float32
    Exp = mybir.ActivationFunctionType.Exp

    nc = bacc.Bacc("TRN2", target_bir_lowering=False, debug=False, num_devices=8)

    xT_d = nc.dram_tensor("xT", [NSTRIP, P, DO, 512], f16,
                          kind="ExternalInput").ap()
    xqT_d = nc.dram_tensor("xqT", [NQSTRIP, P, DO, 512], f16,
                           kind="ExternalInput").ap()
    # one blob for all weights/biases: each early DMA costs ~1.5us of ring
    # turnaround, so the whole critical-path const set ships as one transfer.
    cst_d = nc.dram_tensor("cst", [P, CW + P + 2], f16,
                           kind="ExternalInput").ap()
    msk_d = nc.dram_tensor("msk", [P, NS * 2 * P], f16,
                           kind="ExternalInput").ap()
    out_d = nc.dram_tensor("out", [NQSTRIP, P, 4, P], f16,
                           kind="ExternalOutput").ap()

    with tile.TileContext(nc) as tc:
        with tc.tile_pool(name="const", bufs=1) as cpool, \
             tc.tile_pool(name="data", bufs=1) as dpool:
            warm_sb = cpool.tile([P, 512], f16)
            act_scr = cpool.tile([P, 1], f16)
            cst_sb = cpool.tile([P, CW + P + 2], f16)
            msk_sb = cpool.tile([P, NS * 2 * P], f16)

            def wq_c(o):            # wq/sqrt(H) chunk o, [P, P]
                return cst_sb[:, o * P:(o + 1) * P]

            def wk_c(o):
                return cst_sb[:, D + o * P:D + (o + 1) * P]

            def wv_c(o):
                return cst_sb[:, 2 * D + o * P:2 * D + (o + 1) * P]

            bvb_sb = cst_sb[:, CW:CW + P]          # bv broadcast [P, P]
            bqk_sb = cpool.tile([P, 2], f32)       # fp32 (tensor_scalar req)
            bq_sb = bqk_sb[:, 0:1]
            bk_sb = bqk_sb[:, 1:2]

            # gpsimd only does SBUF memsets: its SWDGE DMA ring is serviced
            # ~10us later than the HWDGE rings, so everything compute needs
            # early goes on the sync/scalar HWDGE queues instead.
            nc.gpsimd.memset(warm_sb[:], 0.0)

            # per-strip tiles so dependencies stay fine-grained: attention on
            # early k-tiles runs while later x strips are still in flight.
            xq_t = [dpool.tile([P, DO, 512], f16, name=f"xq_{i}")
                    for i in range(NQSTRIP)]
            xt_t = [dpool.tile([P, DO, 512], f16, name=f"xt_{i}")
                    for i in range(NSTRIP)]
            qt_t = [dpool.tile([P, 512], f16, name=f"qt_{i}")
                    for i in range(NQSTRIP)]
            kt_t = [dpool.tile([P, 512], f16, name=f"kt_{i}")
                    for i in range(NSTRIP)]
            v_t = [dpool.tile([P, 130], f16, name=f"v_{i}") for i in range(NT)]

            # scalar HWDGE queue: group-0 queries + q-side consts first, then
            # a dummy exp so walrus schedules the ~2.7us ACT table load during
            # the DMA wait. Query strips for groups 1-3 are only consumed in
            # phase 2, so they ride at the back of the ring.
            # scalar HWDGE queue carries ONLY what completes before the first
            # exp: later DMA issues would block the in-order Scalar queue on
            # ring credits and stall the whole ACTIVATE stream behind them.
            nc.scalar.dma_start(cst_sb[:], cst_d)
            nc.scalar.dma_start(xq_t[0][:], xqT_d[0])
            nc.scalar.activation(act_scr[:], warm_sb[:, 0:1], Exp)
            # widen the f16 bias columns once; everything else reads the blob
            nc.vector.tensor_copy(bqk_sb[:], cst_sb[:, CW + P:CW + P + 2])
            # sync HWDGE queue: x strips in k order, then the phase-2 query
            # strips and boundary masks (first needed ~30us in).
            for st in range(NSTRIP):
                nc.sync.dma_start(xt_t[st][:], xT_d[st])
            nc.sync.dma_start(xq_t[1][:], xqT_d[1])
            nc.sync.dma_start(xq_t[2][:], xqT_d[2])
            nc.sync.dma_start(msk_sb[:], msk_d)
            nc.sync.dma_start(xq_t[3][:], xqT_d[3])
            for tt in range(NT):
                nc.gpsimd.memset(v_t[tt][:, 128:129], 1.0)

            # PE warmup: throwaway matmuls on scratch keep the PE busy from
            # the end of the framework preamble so the HAM clock gate opens
            # (2.4 GHz) before the first x strip lands. Alternate two PSUM
            # tiles so consecutive writes pipeline.
            with tc.tile_pool(name="warm", bufs=2, space="PSUM") as wpp:
                wps = [wpp.tile([P, 512], f32, tag=f"w{i}", name=f"warm_{i}")
                       for i in range(2)]
                for i in range(NWARM):
                    nc.tensor.matmul(wps[i % 2], warm_sb[:, 0:128], warm_sb[:],
                                     start=True, stop=True)

            with tc.tile_pool(name="ps_o", bufs=2, space="PSUM") as po_pool, \
                 tc.tile_pool(name="sb_w", bufs=16) as wpool, \
                 tc.tile_pool(name="sb_f", bufs=2) as fpool:

                o_sbg = [fpool.tile([P, 4, P], f16, tag=f"og{g}",
                                    name=f"osb_{g}") for g in range(4)]

                def finalize(g, ci, o_acc):
                    rec = fpool.tile([P, 1], f32, tag="rec",
                                     name=f"rec_{g}_{ci}")
                    nc.vector.reciprocal(rec, o_acc[:, 128:129])
                    nc.vector.tensor_scalar_mul(o_sbg[g][:, ci],
                                                o_acc[:, 0:128], rec)
                    if g == 3:    # tail group: per-slot DMAs drain earlier
                        nc.sync.dma_start(out_d[g, :, ci], o_sbg[g][:, ci])
                    elif ci == 0:  # slots[0] has the largest KC: finishes last
                        nc.sync.dma_start(out_d[g], o_sbg[g])

                def boundary_masks(p_ap_fn, u, slots, w):
                    for ci, s in enumerate(slots[:w]):
                        if u >= KC[s] - 2:
                            i = u - (KC[s] - 2)
                            nc.vector.tensor_mul(
                                p_ap_fn(ci), p_ap_fn(ci),
                                msk_sb[:, ds((2 * s + i) * P, P)])

                def proj_q(pp, st):
                    ps = pp.tile([P, 512], f32, tag="proj", name=f"psq_{st}")
                    for o in range(DO):
                        nc.tensor.matmul(ps, wq_c(o), xq_t[st][:, o],
                                         start=(o == 0), stop=(o == DO - 1))
                    nc.vector.tensor_scalar_add(qt_t[st][:], ps, bq_sb)

                def proj_k(pp, st):
                    ps = pp.tile([P, 512], f32, tag="proj", name=f"psk_{st}")
                    for o in range(DO):
                        nc.tensor.matmul(ps, wk_c(o), xt_t[st][:, o],
                                         start=(o == 0), stop=(o == DO - 1))
                    nc.vector.tensor_scalar_add(kt_t[st][:], ps, bk_sb)

                def proj_v(pp, st):
                    for j in range(4):
                        tt = 4 * st + j
                        ps_v = pp.tile([P, P], f32, tag="proj", name=f"psv_{tt}")
                        for o in range(DO):
                            nc.tensor.matmul(ps_v, xt_t[st][:, o, ts(j, P)],
                                             wv_c(o),
                                             start=(o == 0), stop=(o == DO - 1))
                        nc.vector.tensor_add(v_t[tt][:, 0:128], ps_v, bvb_sb)

                # ---- phase 1: projections with group 0 interleaved ----
                slots0 = [0, 1, 2, 3]
                ob0 = {i: po_pool.tile([P, 2, 129], f32, tag="oacc",
                                       name=f"o_bank_0_{i}") for i in range(2)}
                o_ps0 = {s: ob0[ci // 2][:, ci % 2]
                         for ci, s in enumerate(slots0)}
                first0 = {0, 2}
                with tc.tile_pool(name="pproj", bufs=2, space="PSUM") as pp, \
                     tc.tile_pool(name="ps_s0", bufs=2, space="PSUM") as ps0:
                    for st in range(NSTRIP):
                        # group-0 queries immediately; groups 1-3 only when
                        # their (ring-tail) xq strips have landed, and late
                        # enough that the in-order PE queue never stalls.
                        if st == 0:
                            proj_q(pp, 0)
                        elif st >= 5:
                            proj_q(pp, st - 4)
                        # K first, then the strip's S matmuls + exp so the
                        # ScalarE stream starts while V is still projecting.
                        # Strips 6-7's group-0 attention (u >= 24) runs in the
                        # phase-2 triple pipeline instead, shortening phase 1.
                        proj_k(pp, st)
                        pdat = []
                        for up in ((2 * st, 2 * st + 1) if st < 6 else ()):
                            u0 = 2 * up
                            w = sum(1 for s in slots0 if KC[s] > u0)
                            s_sp = ps0.tile([P, 2, 512], f32, tag="s0",
                                            name=f"s0_{u0}")
                            for j in range(2):
                                u = u0 + j
                                nc.tensor.matmul(s_sp[:, j, 0:w * P],
                                                 kt_t[st][:, ts(u % 4, P)],
                                                 qt_t[0][:, 0:w * P],
                                                 start=True, stop=True)
                            p_sb = wpool.tile([P, 2, 512], f16, tag="ptile0",
                                              name=f"p0_{u0}")
                            nc.scalar.activation(p_sb[:, :, 0:w * P],
                                                 s_sp[:, :, 0:w * P], Exp)
                            pdat.append((u0, w, p_sb))
                        proj_v(pp, st)
                        for u0, w, p_sb in pdat:
                            for j in range(2):
                                boundary_masks(
                                    lambda ci, j=j: p_sb[:, j, ts(ci, P)],
                                    u0 + j, slots0, w)
                            for j in range(2):
                                u = u0 + j
                                for ci, s in enumerate(slots0[:w]):
                                    nc.tensor.matmul(
                                        o_ps0[s], p_sb[:, j, ts(ci, P)],
                                        v_t[u][:, 0:129],
                                        start=(u == 0 and s in first0),
                                        stop=(u == KC[s] - 1),
                                        skip_group_check=True)
                                    if u == KC[s] - 1:
                                        finalize(0, ci, o_ps0[s])

                # ---- phase 2: attention, three k-tiles per exp ----
                # ScalarE has a ~293 ns fixed cost per ACTIVATE; triple-wide S
                # tiles amortize it. PSUM budget: S 3 banks x 2 bufs + two O
                # accumulators packed per bank (2 banks) = 8. Packing relies
                # on per-element has_written: the bank's first PV (slot A,
                # u=0, start=True) clears the bank; slot B's u=0 PV uses
                # start=False and overwrites its still-unwritten elements.
                with tc.tile_pool(name="ps_s", bufs=2, space="PSUM") as ps_pool:
                    items = [(0, slots0, 24, 3), (0, slots0, 27, 3),
                             (0, slots0, 30, 2)]
                    for g in range(1, 4):
                        slots = list(range(4 * g, 4 * g + 4))
                        for u0 in range(0, KC[slots[0]], 3):
                            items.append((g, slots, u0,
                                          min(3, KC[slots[0]] - u0)))

                    o_banks = {0: o_ps0}

                    def group_o(g, slots):
                        if g not in o_banks:
                            ob = {i: po_pool.tile([P, 2, 129], f32,
                                                  tag="oacc",
                                                  name=f"o_bank_{g}_{i}")
                                  for i in range(2)}
                            o_banks[g] = {s: ob[ci // 2][:, ci % 2]
                                          for ci, s in enumerate(slots)}
                        return o_banks[g]

                    def emit_s(item):
                        g, slots, u0, span = item
                        s_ps = ps_pool.tile([P, 3, 512], f32, tag="sacc",
                                            name=f"s_{g}_{u0}")
                        for j in range(span):
                            u = u0 + j
                            wj = sum(1 for s in slots if KC[s] > u)
                            nc.tensor.matmul(s_ps[:, j, 0:wj * P],
                                             kt_t[u // 4][:, ts(u % 4, P)],
                                             qt_t[g][:, 0:wj * P],
                                             start=True, stop=True)
                        return s_ps

                    # software-pipelined: the next item's S matmuls are
                    # emitted right after this item's exp, so the PE fills
                    # the ACT latency (also across group transitions).
                    s_cur = emit_s(items[0])
                    for idx, item in enumerate(items):
                        g, slots, u0, span = item
                        o_ps = group_o(g, slots)
                        first_in_bank = {slots[0], slots[2]}
                        w0 = sum(1 for s in slots if KC[s] > u0)
                        p_sb = wpool.tile([P, 3, 512], f16, tag="ptile",
                                          name=f"p_{g}_{u0}")
                        nc.scalar.activation(p_sb[:, 0:span, 0:w0 * P],
                                             s_cur[:, 0:span, 0:w0 * P],
                                             Exp)
                        if idx + 1 < len(items):
                            s_cur = emit_s(items[idx + 1])
                        for j in range(span):
                            boundary_masks(
                                lambda ci, j=j: p_sb[:, j, ts(ci, P)],
                                u0 + j, slots,
                                sum(1 for s in slots if KC[s] > u0 + j))
                        for j in range(span):
                            u = u0 + j
                            wj = sum(1 for s in slots if KC[s] > u)
                            for ci, s in enumerate(slots[:wj]):
                                nc.tensor.matmul(
                                    o_ps[s], p_sb[:, j, ts(ci, P)],
                                    v_t[u][:, 0:129],
                                    start=(u == 0 and s in first_in_bank),
                                    stop=(u == KC[s] - 1),
                                    skip_group_check=True)
                                if u == KC[s] - 1:
                                    finalize(g, ci, o_ps[s])

    nc.compile()
    return nc


def _prep_core(core, x, wq, bq, wk, bk, wv, bv):
    b, ct = core // 2, core % 2
    qtiles = _slot_qtile(ct)
    scale = np.float32(1.0 / np.sqrt(H))

    def dchunk(a):  # [D, N] -> [P, DO, N] with d = o*P + p
        return np.ascontiguousarray(
            a.reshape(DO, P, -1).transpose(1, 0, 2)).astype(_f16)

    def stripmajor(a, nstrip):  # [P, DO, N] -> [nstrip, P, DO, 512]
        return np.ascontiguousarray(
            a.reshape(P, DO, nstrip, 512).transpose(2, 0, 1, 3))

    xT = x[b].T.astype(np.float32)                      # [D, T]
    qrows = np.concatenate([np.arange(j * P, (j + 1) * P) for j in qtiles])
    xqT = np.ascontiguousarray(xT[:, qrows])            # [D, TQ]

    # per-block boundary mask: ones below the causal edge, triu on the
    # diagonal tile, zeros for schedule-padding tiles past the edge.
    msk = np.zeros((P, NS * 2 * P), dtype=_f16)
    triu = np.triu(np.ones((P, P), dtype=_f16))
    for s in range(NS):
        j = qtiles[s]
        for i in range(2):
            u = KC[s] - 2 + i
            blk = 2 * s + i
            if u < j:
                msk[:, blk * P:(blk + 1) * P] = 1.0
            elif u == j:
                msk[:, blk * P:(blk + 1) * P] = triu

    cst = np.concatenate([
        dchunk(wq * scale).reshape(P, D).astype(np.float32),
        dchunk(wk).reshape(P, D).astype(np.float32),
        dchunk(wv).reshape(P, D).astype(np.float32),
        np.tile(bv.astype(np.float32), (P, 1)),
        (bq * scale).astype(np.float32).reshape(P, 1),
        bk.astype(np.float32).reshape(P, 1),
    ], axis=1)

    return {
        "xT": stripmajor(dchunk(xT), NSTRIP),
        "xqT": stripmajor(dchunk(xqT), NQSTRIP),
        "cst": cst.astype(_f16),
        "msk": msk,
    }


def _fallback(x, mask, wq, bq, wk, bk, wv, bv):
    """Exact numpy path for inputs the specialized kernel doesn't cover."""
    out = np.empty((x.shape[0], x.shape[1], wq.shape[1]), dtype=np.float32)
    scale = np.float32(1.0 / np.sqrt(wq.shape[1]))
    for b in range(x.shape[0]):
        q = x[b] @ wq + bq
        k = x[b] @ wk + bk
        v = x[b] @ wv + bv
        s = (q @ k.T) * scale
        s = np.where(mask == 0, np.float32(-1e30), s)
        s -= s.max(axis=-1, keepdims=True)
        p = np.exp(s)
        p /= p.sum(axis=-1, keepdims=True)
        out[b] = p @ v
    return out


def kernel(**inputs):
    x = np.asarray(inputs["x"], dtype=np.float32)
    mask = np.asarray(inputs["mask"])
    wq = np.asarray(inputs["wq"], dtype=np.float32)
    bq = np.asarray(inputs["bq"], dtype=np.float32)
    wk = np.asarray(inputs["wk"], dtype=np.float32)
    bk = np.asarray(inputs["bk"], dtype=np.float32)
    wv = np.asarray(inputs["wv"], dtype=np.float32)
    bv = np.asarray(inputs["bv"], dtype=np.float32)

    causal = (x.shape == (B, T, D) and wq.shape == (D, H)
              and np.array_equal(mask, np.tril(np.ones((T, T), mask.dtype))))
    if not causal:
        return _fallback(x, mask, wq, bq, wk, bk, wv, bv)

    if "nc" not in _CACHE:
        _CACHE["nc"] = _build_program()
    nc = _CACHE["nc"]

    from concourse import bass_utils
    in_maps = [_prep_core(c, x, wq, bq, wk, bk, wv, bv) for c in range(8)]
    res = bass_utils.run_bass_kernel_spmd(nc, in_maps, core_ids=list(range(8)),
                                          **_CACHE.get("run_kwargs", {}))
    _CACHE["last_result"] = res

    out = np.empty((B, T, H), dtype=np.float32)
    for c in range(8):
        b, ct = c // 2, c % 2
        qtiles = _slot_qtile(ct)
        oc = res.results[c]["out"].astype(np.float32)   # [NQSTRIP, P, 4, P]
        for s, j in enumerate(qtiles):
            out[b, j * P:(j + 1) * P, :] = oc[s // 4, :, s % 4]
    return out


# revision 54
# speedup vs baseline: 1.0474x; 1.0474x over previous
"""Single-head causal attention (B=4, T=4096, D=512, H=128) on 8 TRN2 NeuronCores.

Sharding: data-parallel over batch (4 batches x 2 cores). The two cores of a
batch split the 32 query tiles zig-zag style so causal work is balanced
(each core gets one long-context and one short-context tile per pair).
One SPMD program serves both core "types": per-slot k-tile counts are padded
to a shared compile-time schedule, and the causal boundary is applied with
per-core 0/1 mask tiles supplied as input data (host precomputed, DMA'd).

Per-core device program (all matmuls fp16 inputs -> fp32 PSUM):
  K^T = (wk^T @ x^T), V = x @ wv + bv, Q^T = (wq'^T @ xq^T), wq' = wq/sqrt(H)
  per slot group (4 slots, k-outer): S^T[k,q] = K^T_tile.T @ Q^T block
  P = exp(S^T) (no max subtraction: scores are O(5), fp16 holds exp fine),
  boundary tiles multiplied by 0/1 masks, O[q, 0:129] += P^T.T @ [V | 1]
  out = O[:, :128] * (1 / O[:, 128]).
The ones-column of V carries the softmax denominator through the same PSUM
accumulation, so no running max/sum bookkeeping is needed.

Startup engineering (v2): strip-major DRAM layout gives 4KB-contiguous DMA
lines per partition; weights go out first on the gpsimd (SWDGE) queue while
x strips stream on the two HWDGE queues; a dummy exp preloads the ScalarE
activation table set; and a short chain of throwaway matmuls on scratch SBUF
warms the PE HAM clock gate (1.2 -> 2.4 GHz) before real work arrives.
"""

import numpy as np
import ml_dtypes

B, T, D, H = 4, 4096, 512, 128
P = 128          # partitions / tile edge
DO = D // P      # contraction chunks (4)
NT = T // P      # k tiles per batch (32)
NS = 16          # query-tile slots per core
TQ = NS * P      # queries per core (2048)
KC = [32 - 2 * s for s in range(NS)]   # k-tiles processed per slot (desc)
NSTRIP = T // 512          # 8 key strips
NQSTRIP = TQ // 512        # 4 query strips (one per slot group)
NWARM = 6                  # HAM warmup matmuls
CW = 3 * D                 # packed const blob: wq|wk|wv chunks (f16 cols)

_f16 = np.float16

_CACHE = {}


def _slot_qtile(core_type: int):
    """Global q-tile index handled by each slot, for core type 0/1."""
    out = []
    for s in range(NS):
        if s < 8:
            j = 31 - 2 * s - core_type          # long-context slots
        else:
            p = 15 - s
            j = 2 * p + core_type               # short-context slots
        out.append(j)
    return out


def _build_program():
    import concourse.tile as tile
    from concourse import bacc, mybir
    from concourse.bass import ts, ds

    f16 = mybir.dt.float16
    f32 = mybir.dt.float32
    Exp = mybir.ActivationFunctionType.Exp

    nc = bacc.Bacc("TRN2", target_bir_lowering=False, debug=False, num_devices=8)

    xT_d = nc.dram_tensor("xT", [NSTRIP, P, DO, 512], f16,
                          kind="ExternalInput").ap()
    xqT_d = nc.dram_tensor("xqT", [NQSTRIP, P, DO, 512], f16,
                           kind="ExternalInput").ap()
    # one blob for all weights/biases: each early DMA costs ~1.5us of ring
    # turnaround, so the whole critical-path const set ships as one transfer.
    cst_d = nc.dram_tensor("cst", [P, CW + P + 2], f16,
                           kind="ExternalInput").ap()
    msk_d = nc.dram_tensor("msk", [P, NS * 2 * P], f16,
                           kind="ExternalInput").ap()
    out_d = nc.dram_tensor("out", [NQSTRIP, P, 4, P], f16,
                           kind="ExternalOutput").ap()

    with tile.TileContext(nc) as tc:
        with tc.tile_pool(name="const", bufs=1) as cpool, \
             tc.tile_pool(name="data", bufs=1) as dpool:
            warm_sb = cpool.tile([P, 512], f16)
            act_scr = cpool.tile([P, 1], f16)
            cst_sb = cpool.tile([P, CW + P + 2], f16)
            msk_sb = cpool.tile([P, NS * 2 * P], f16)

            def wq_c(o):            # wq/sqrt(H) chunk o, [P, P]
                return cst_sb[:, o * P:(o + 1) * P]

            def wk_c(o):
                return cst_sb[:, D + o * P:D + (o + 1) * P]

            def wv_c(o):
                return cst_sb[:, 2 * D + o * P:2 * D + (o + 1) * P]

            bvb_sb = cst_sb[:, CW:CW + P]          # bv broadcast [P, P]
            bqk_sb = cpool.tile([P, 2], f32)       # fp32 (tensor_scalar req)
            bq_sb = bqk_sb[:, 0:1]
            bk_sb = bqk_sb[:, 1:2]

            # gpsimd only does SBUF memsets: its SWDGE DMA ring is serviced
            # ~10us later than the HWDGE rings, so everything compute needs
            # early goes on the sync/scalar HWDGE queues instead.
            nc.gpsimd.memset(warm_sb[:], 0.0)

            # per-strip tiles so dependencies stay fine-grained: attention on
            # early k-tiles runs while later x strips are still in flight.
            xq_t = [dpool.tile([P, DO, 512], f16, name=f"xq_{i}")
                    for i in range(NQSTRIP)]
            xt_t = [dpool.tile([P, DO, 512], f16, name=f"xt_{i}")
                    for i in range(NSTRIP)]
            qt_t = [dpool.tile([P, 512], f16, name=f"qt_{i}")
                    for i in range(NQSTRIP)]
            kt_t = [dpool.tile([P, 512], f16, name=f"kt_{i}")
                    for i in range(NSTRIP)]
            v_t = [dpool.tile([P, 130], f16, name=f"v_{i}") for i in range(NT)]

            # scalar HWDGE queue: group-0 queries + q-side consts first, then
            # a dummy exp so walrus schedules the ~2.7us ACT table load during
            # the DMA wait. Query strips for groups 1-3 are only consumed in
            # phase 2, so they ride at the back of the ring.
            # scalar HWDGE queue carries ONLY what completes before the first
            # exp: later DMA issues would block the in-order Scalar queue on
            # ring credits and stall the whole ACTIVATE stream behind them.
            # the scalar ring is ~2x slower per transfer than sync; it gets
            # ONLY xq0 (so it lands in parallel with cst+xt0 on sync) and the
            # table-load dummy. Anything more would also block the in-order
            # Scalar queue (ring credits) and stall the ACTIVATE stream.
            nc.scalar.dma_start(xq_t[0][:], xqT_d[0])
            nc.scalar.activation(act_scr[:], warm_sb[:, 0:1], Exp)
            # sync HWDGE queue: const blob, x strips in k order, then the
            # phase-2 query strips and boundary masks (first needed ~30us in).
            nc.sync.dma_start(cst_sb[:], cst_d)
            for st in range(NSTRIP):
                nc.sync.dma_start(xt_t[st][:], xT_d[st])
            nc.sync.dma_start(xq_t[1][:], xqT_d[1])
            nc.sync.dma_start(xq_t[2][:], xqT_d[2])
            nc.sync.dma_start(msk_sb[:], msk_d)
            nc.sync.dma_start(xq_t[3][:], xqT_d[3])
            # widen the f16 bias columns once; everything else reads the blob
            nc.vector.tensor_copy(bqk_sb[:], cst_sb[:, CW + P:CW + P + 2])
            for tt in range(NT):
                nc.gpsimd.memset(v_t[tt][:, 128:129], 1.0)

            # PE warmup: throwaway matmuls on scratch keep the PE busy from
            # the end of the framework preamble so the HAM clock gate opens
            # (2.4 GHz) before the first x strip lands. Alternate two PSUM
            # tiles so consecutive writes pipeline.
            with tc.tile_pool(name="warm", bufs=2, space="PSUM") as wpp:
                wps = [wpp.tile([P, 512], f32, tag=f"w{i}", name=f"warm_{i}")
                       for i in range(2)]
                for i in range(NWARM):
                    nc.tensor.matmul(wps[i % 2], warm_sb[:, 0:128], warm_sb[:],
                                     start=True, stop=True)

            with tc.tile_pool(name="ps_o", bufs=2, space="PSUM") as po_pool, \
                 tc.tile_pool(name="sb_w", bufs=16) as wpool, \
                 tc.tile_pool(name="sb_f", bufs=2) as fpool:

                o_sbg = [fpool.tile([P, 4, P], f16, tag=f"og{g}",
                                    name=f"osb_{g}") for g in range(4)]

                def finalize(g, ci, o_acc):
                    rec = fpool.tile([P, 1], f32, tag="rec",
                                     name=f"rec_{g}_{ci}")
                    nc.vector.reciprocal(rec, o_acc[:, 128:129])
                    nc.vector.tensor_scalar_mul(o_sbg[g][:, ci],
                                                o_acc[:, 0:128], rec)
                    if g == 3:    # tail group: per-slot DMAs drain earlier
                        nc.sync.dma_start(out_d[g, :, ci], o_sbg[g][:, ci])
                    elif ci == 0:  # slots[0] has the largest KC: finishes last
                        nc.sync.dma_start(out_d[g], o_sbg[g])

                def boundary_masks(p_ap_fn, u, slots, w):
                    for ci, s in enumerate(slots[:w]):
                        if u >= KC[s] - 2:
                            i = u - (KC[s] - 2)
                            nc.vector.tensor_mul(
                                p_ap_fn(ci), p_ap_fn(ci),
                                msk_sb[:, ds((2 * s + i) * P, P)])

                def proj_q(pp, st):
                    ps = pp.tile([P, 512], f32, tag="proj", name=f"psq_{st}")
                    for o in range(DO):
                        nc.tensor.matmul(ps, wq_c(o), xq_t[st][:, o],
                                         start=(o == 0), stop=(o == DO - 1))
                    nc.vector.tensor_scalar_add(qt_t[st][:], ps, bq_sb)

                def proj_k(pp, st):
                    ps = pp.tile([P, 512], f32, tag="proj", name=f"psk_{st}")
                    for o in range(DO):
                        nc.tensor.matmul(ps, wk_c(o), xt_t[st][:, o],
                                         start=(o == 0), stop=(o == DO - 1))
                    nc.vector.tensor_scalar_add(kt_t[st][:], ps, bk_sb)

                def proj_v(pp, st):
                    for j in range(4):
                        tt = 4 * st + j
                        ps_v = pp.tile([P, P], f32, tag="proj", name=f"psv_{tt}")
                        for o in range(DO):
                            nc.tensor.matmul(ps_v, xt_t[st][:, o, ts(j, P)],
                                             wv_c(o),
                                             start=(o == 0), stop=(o == DO - 1))
                        nc.vector.tensor_add(v_t[tt][:, 0:128], ps_v, bvb_sb)

                # ---- phase 1: projections with group 0 interleaved ----
                slots0 = [0, 1, 2, 3]
                ob0 = {i: po_pool.tile([P, 2, 129], f32, tag="oacc",
                                       name=f"o_bank_0_{i}") for i in range(2)}
                o_ps0 = {s: ob0[ci // 2][:, ci % 2]
                         for ci, s in enumerate(slots0)}
                first0 = {0, 2}
                with tc.tile_pool(name="pproj", bufs=2, space="PSUM") as pp, \
                     tc.tile_pool(name="ps_s0", bufs=2, space="PSUM") as ps0:
                    for st in range(NSTRIP):
                        # group-0 queries immediately; groups 1-3 only when
                        # their (ring-tail) xq strips have landed, and late
                        # enough that the in-order PE queue never stalls.
                        if st == 0:
                            proj_q(pp, 0)
                        elif st >= 5:
                            proj_q(pp, st - 4)
                        # K first, then the strip's S matmuls + exp so the
                        # ScalarE stream starts while V is still projecting.
                        proj_k(pp, st)
                        pdat = []
                        for up in (2 * st, 2 * st + 1):   # g0 pairs
                            u0 = 2 * up
                            w = sum(1 for s in slots0 if KC[s] > u0)
                            s_sp = ps0.tile([P, 2, 512], f32, tag="s0",
                                            name=f"s0_{u0}")
                            for j in range(2):
                                u = u0 + j
                                nc.tensor.matmul(s_sp[:, j, 0:w * P],
                                                 kt_t[st][:, ts(u % 4, P)],
                                                 qt_t[0][:, 0:w * P],
                                                 start=True, stop=True)
                            p_sb = wpool.tile([P, 2, 512], f16, tag="ptile0",
                                              name=f"p0_{u0}")
                            nc.scalar.activation(p_sb[:, :, 0:w * P],
                                                 s_sp[:, :, 0:w * P], Exp)
                            pdat.append((u0, w, p_sb))
                        proj_v(pp, st)
                        for u0, w, p_sb in pdat:
                            for j in range(2):
                                boundary_masks(
                                    lambda ci, j=j: p_sb[:, j, ts(ci, P)],
                                    u0 + j, slots0, w)
                            for j in range(2):
                                u = u0 + j
                                for ci, s in enumerate(slots0[:w]):
                                    nc.tensor.matmul(
                                        o_ps0[s], p_sb[:, j, ts(ci, P)],
                                        v_t[u][:, 0:129],
                                        start=(u == 0 and s in first0),
                                        stop=(u == KC[s] - 1),
                                        skip_group_check=True)
                                    if u == KC[s] - 1:
                                        finalize(0, ci, o_ps0[s])

                # ---- phase 2: attention, three k-tiles per exp ----
                # ScalarE has a ~293 ns fixed cost per ACTIVATE; triple-wide S
                # tiles amortize it. PSUM budget: S 3 banks x 2 bufs + two O
                # accumulators packed per bank (2 banks) = 8. Packing relies
                # on per-element has_written: the bank's first PV (slot A,
                # u=0, start=True) clears the bank; slot B's u=0 PV uses
                # start=False and overwrites its still-unwritten elements.
                with tc.tile_pool(name="ps_s", bufs=2, space="PSUM") as ps_pool:
                    items = []
                    for g in range(1, 4):
                        slots = list(range(4 * g, 4 * g + 4))
                        for u0 in range(0, KC[slots[0]], 3):
                            items.append((g, slots, u0,
                                          min(3, KC[slots[0]] - u0)))

                    o_banks = {}

                    def group_o(g, slots):
                        if g not in o_banks:
                            ob = {i: po_pool.tile([P, 2, 129], f32,
                                                  tag="oacc",
                                                  name=f"o_bank_{g}_{i}")
                                  for i in range(2)}
                            o_banks[g] = {s: ob[ci // 2][:, ci % 2]
                                          for ci, s in enumerate(slots)}
                        return o_banks[g]

                    def emit_s(item):
                        g, slots, u0, span = item
                        s_ps = ps_pool.tile([P, 3, 512], f32, tag="sacc",
                                            name=f"s_{g}_{u0}")
                        for j in range(span):
                            u = u0 + j
                            wj = sum(1 for s in slots if KC[s] > u)
                            nc.tensor.matmul(s_ps[:, j, 0:wj * P],
                                             kt_t[u // 4][:, ts(u % 4, P)],
                                             qt_t[g][:, 0:wj * P],
                                             start=True, stop=True)
                        return s_ps

                    # software-pipelined: the next item's S matmuls are
                    # emitted right after this item's exp, so the PE fills
                    # the ACT latency (also across group transitions).
                    s_cur = emit_s(items[0])
                    for idx, item in enumerate(items):
                        g, slots, u0, span = item
                        o_ps = group_o(g, slots)
                        first_in_bank = {slots[0], slots[2]}
                        w0 = sum(1 for s in slots if KC[s] > u0)
                        p_sb = wpool.tile([P, 3, 512], f16, tag="ptile",
                                          name=f"p_{g}_{u0}")
                        nc.scalar.activation(p_sb[:, 0:span, 0:w0 * P],
                                             s_cur[:, 0:span, 0:w0 * P],
                                             Exp)
                        if idx + 1 < len(items):
                            s_cur = emit_s(items[idx + 1])
                        for j in range(span):
                            boundary_masks(
                                lambda ci, j=j: p_sb[:, j, ts(ci, P)],
                                u0 + j, slots,
                                sum(1 for s in slots if KC[s] > u0 + j))
                        for j in range(span):
                            u = u0 + j
                            wj = sum(1 for s in slots if KC[s] > u)
                            for ci, s in enumerate(slots[:wj]):
                                nc.tensor.matmul(
                                    o_ps[s], p_sb[:, j, ts(ci, P)],
                                    v_t[u][:, 0:129],
                                    start=(u == 0 and s in first_in_bank),
                                    stop=(u == KC[s] - 1),
                                    skip_group_check=True)
                                if u == KC[s] - 1:
                                    finalize(g, ci, o_ps[s])

    nc.compile()
    return nc


def _prep_core(core, x, wq, bq, wk, bk, wv, bv):
    b, ct = core // 2, core % 2
    qtiles = _slot_qtile(ct)
    scale = np.float32(1.0 / np.sqrt(H))

    def dchunk(a):  # [D, N] -> [P, DO, N] with d = o*P + p
        return np.ascontiguousarray(
            a.reshape(DO, P, -1).transpose(1, 0, 2)).astype(_f16)

    def stripmajor(a, nstrip):  # [P, DO, N] -> [nstrip, P, DO, 512]
        return np.ascontiguousarray(
            a.reshape(P, DO, nstrip, 512).transpose(2, 0, 1, 3))

    xT = x[b].T.astype(np.float32)                      # [D, T]
    qrows = np.concatenate([np.arange(j * P, (j + 1) * P) for j in qtiles])
    xqT = np.ascontiguousarray(xT[:, qrows])            # [D, TQ]

    # per-block boundary mask: ones below the causal edge, triu on the
    # diagonal tile, zeros for schedule-padding tiles past the edge.
    msk = np.zeros((P, NS * 2 * P), dtype=_f16)
    triu = np.triu(np.ones((P, P), dtype=_f16))
    for s in range(NS):
        j = qtiles[s]
        for i in range(2):
            u = KC[s] - 2 + i
            blk = 2 * s + i
            if u < j:
                msk[:, blk * P:(blk + 1) * P] = 1.0
            elif u == j:
                msk[:, blk * P:(blk + 1) * P] = triu

    cst = np.concatenate([
        dchunk(wq * scale).reshape(P, D).astype(np.float32),
        dchunk(wk).reshape(P, D).astype(np.float32),
        dchunk(wv).reshape(P, D).astype(np.float32),
        np.tile(bv.astype(np.float32), (P, 1)),
        (bq * scale).astype(np.float32).reshape(P, 1),
        bk.astype(np.float32).reshape(P, 1),
    ], axis=1)

    return {
        "xT": stripmajor(dchunk(xT), NSTRIP),
        "xqT": stripmajor(dchunk(xqT), NQSTRIP),
        "cst": cst.astype(_f16),
        "msk": msk,
    }


def _fallback(x, mask, wq, bq, wk, bk, wv, bv):
    """Exact numpy path for inputs the specialized kernel doesn't cover."""
    out = np.empty((x.shape[0], x.shape[1], wq.shape[1]), dtype=np.float32)
    scale = np.float32(1.0 / np.sqrt(wq.shape[1]))
    for b in range(x.shape[0]):
        q = x[b] @ wq + bq
        k = x[b] @ wk + bk
        v = x[b] @ wv + bv
        s = (q @ k.T) * scale
        s = np.where(mask == 0, np.float32(-1e30), s)
        s -= s.max(axis=-1, keepdims=True)
        p = np.exp(s)
        p /= p.sum(axis=-1, keepdims=True)
        out[b] = p @ v
    return out


def kernel(**inputs):
    x = np.asarray(inputs["x"], dtype=np.float32)
    mask = np.asarray(inputs["mask"])
    wq = np.asarray(inputs["wq"], dtype=np.float32)
    bq = np.asarray(inputs["bq"], dtype=np.float32)
    wk = np.asarray(inputs["wk"], dtype=np.float32)
    bk = np.asarray(inputs["bk"], dtype=np.float32)
    wv = np.asarray(inputs["wv"], dtype=np.float32)
    bv = np.asarray(inputs["bv"], dtype=np.float32)

    causal = (x.shape == (B, T, D) and wq.shape == (D, H)
              and np.array_equal(mask, np.tril(np.ones((T, T), mask.dtype))))
    if not causal:
        return _fallback(x, mask, wq, bq, wk, bk, wv, bv)

    if "nc" not in _CACHE:
        _CACHE["nc"] = _build_program()
    nc = _CACHE["nc"]

    from concourse import bass_utils
    in_maps = [_prep_core(c, x, wq, bq, wk, bk, wv, bv) for c in range(8)]
    res = bass_utils.run_bass_kernel_spmd(nc, in_maps, core_ids=list(range(8)),
                                          **_CACHE.get("run_kwargs", {}))
    _CACHE["last_result"] = res

    out = np.empty((B, T, H), dtype=np.float32)
    for c in range(8):
        b, ct = c // 2, c % 2
        qtiles = _slot_qtile(ct)
        oc = res.results[c]["out"].astype(np.float32)   # [NQSTRIP, P, 4, P]
        for s, j in enumerate(qtiles):
            out[b, j * P:(j + 1) * P, :] = oc[s // 4, :, s % 4]
    return out


# revision 55
# speedup vs baseline: 1.0883x; 1.0390x over previous
"""Single-head causal attention (B=4, T=4096, D=512, H=128) on 8 TRN2 NeuronCores.

Sharding: data-parallel over batch (4 batches x 2 cores). The two cores of a
batch split the 32 query tiles zig-zag style so causal work is balanced
(each core gets one long-context and one short-context tile per pair).
One SPMD program serves both core "types": per-slot k-tile counts are padded
to a shared compile-time schedule, and the causal boundary is applied with
per-core 0/1 mask tiles built on-device from host-supplied (a,b) codes.

Per-core device program (all matmuls fp16 inputs -> fp32 PSUM):
  K^T = (wk^T @ x^T), V = x @ wv + bv, Q^T = (wq'^T @ xq^T), wq' = wq/sqrt(H)
  per slot group (4 slots, k-outer): S^T[k,q] = K^T_tile.T @ Q^T block
  P = exp(S^T) (no max subtraction: scores are O(5), fp16 holds exp fine),
  boundary tiles multiplied by 0/1 masks, O[q, 0:129] += P^T.T @ [V | 1]
  out = O[:, :128] * (1 / O[:, 128]).
The ones-column of V carries the softmax denominator through the same PSUM
accumulation, so no running max/sum bookkeeping is needed.

Startup engineering: strip-major DRAM layout gives 4KB-contiguous DMA lines;
weights lead the sync ring ahead of the x strips while the query strips ride
the scalar ring; a dummy exp preloads the ScalarE activation table set; and
a short chain of throwaway matmuls on scratch SBUF warms the PE HAM clock
gate (1.2 -> 2.4 GHz) before real work arrives.
"""

import numpy as np
import ml_dtypes

B, T, D, H = 4, 4096, 512, 128
P = 128          # partitions / tile edge
DO = D // P      # contraction chunks (4)
NT = T // P      # k tiles per batch (32)
NS = 16          # query-tile slots per core
TQ = NS * P      # queries per core (2048)
KC = [32 - 2 * s for s in range(NS)]   # k-tiles processed per slot (desc)
NSTRIP = T // 512          # 8 key strips
NQSTRIP = TQ // 512        # 4 query strips (one per slot group)
NWARM = 10                 # HAM warmup matmuls

_f16 = np.float16

_CACHE = {}


def _slot_qtile(core_type: int):
    """Global q-tile index handled by each slot, for core type 0/1."""
    out = []
    for s in range(NS):
        if s < 8:
            j = 31 - 2 * s - core_type          # long-context slots
        else:
            p = 15 - s
            j = 2 * p + core_type               # short-context slots
        out.append(j)
    return out


def _build_program():
    import concourse.tile as tile
    from concourse import bacc, mybir
    from concourse.bass import ts, ds

    f16 = mybir.dt.float16
    f32 = mybir.dt.float32
    Exp = mybir.ActivationFunctionType.Exp

    nc = bacc.Bacc("TRN2", target_bir_lowering=False, debug=False, num_devices=8)

    xT_d = nc.dram_tensor("xT", [NSTRIP, P, DO, 512], f16,
                          kind="ExternalInput").ap()
    xqT_d = nc.dram_tensor("xqT", [NQSTRIP, P, DO, 512], f16,
                           kind="ExternalInput").ap()
    wq_d = nc.dram_tensor("wq", [P, DO, P], f16, kind="ExternalInput").ap()
    wk_d = nc.dram_tensor("wk", [P, DO, P], f16, kind="ExternalInput").ap()
    wv_d = nc.dram_tensor("wv", [P, DO, P], f16, kind="ExternalInput").ap()
    bq_d = nc.dram_tensor("bq", [P, 1], f32, kind="ExternalInput").ap()
    bk_d = nc.dram_tensor("bk", [P, 1], f32, kind="ExternalInput").ap()
    bvb_d = nc.dram_tensor("bvb", [P, P], f32, kind="ExternalInput").ap()
    tril_d = nc.dram_tensor("tril", [P, P], f16, kind="ExternalInput").ap()
    ab_d = nc.dram_tensor("ab", [P, NS * 2 * 2], f32, kind="ExternalInput").ap()
    out_d = nc.dram_tensor("out", [NQSTRIP, P, 4, P], f16,
                           kind="ExternalOutput").ap()

    with tile.TileContext(nc) as tc:
        with tc.tile_pool(name="const", bufs=1) as cpool, \
             tc.tile_pool(name="data", bufs=1) as dpool:
            warm_sb = cpool.tile([P, 512], f16)
            act_scr = cpool.tile([P, 1], f16)
            wq_sb = cpool.tile([P, DO, P], f16)
            wk_sb = cpool.tile([P, DO, P], f16)
            wv_sb = cpool.tile([P, DO, P], f16)
            bq_sb = cpool.tile([P, 1], f32)
            bk_sb = cpool.tile([P, 1], f32)
            bvb_sb = cpool.tile([P, P], f32)
            msk_sb = cpool.tile([P, NS * 2 * P], f16)
            tril_sb = cpool.tile([P, P], f16)
            ab_sb = cpool.tile([P, NS * 2 * 2], f32)

            nc.gpsimd.memset(warm_sb[:], 0.0)

            xq_t = [dpool.tile([P, DO, 512], f16, name=f"xq_{i}")
                    for i in range(NQSTRIP)]
            xt_t = [dpool.tile([P, DO, 512], f16, name=f"xt_{i}")
                    for i in range(NSTRIP)]
            qt_t = [dpool.tile([P, 512], f16, name=f"qt_{i}")
                    for i in range(NQSTRIP)]
            kt_t = [dpool.tile([P, 512], f16, name=f"kt_{i}")
                    for i in range(NSTRIP)]
            v_t = [dpool.tile([P, 130], f16, name=f"v_{i}") for i in range(NT)]

            # scalar HWDGE queue: group-0 queries + q-side consts first, a
            # dummy exp so the ~2.7us ACT table load happens during the DMA
            # wait, then the small mask consts and remaining query strips.
            nc.scalar.dma_start(xq_t[0][:], xqT_d[0])
            nc.scalar.dma_start(wq_sb[:], wq_d)
            nc.scalar.dma_start(bq_sb[:], bq_d)
            nc.scalar.activation(act_scr[:], warm_sb[:, 0:1], Exp)
            nc.scalar.dma_start(tril_sb[:], tril_d)
            nc.scalar.dma_start(ab_sb[:], ab_d)
            for g in range(1, NQSTRIP):
                nc.scalar.dma_start(xq_t[g][:], xqT_d[g])
            # sync HWDGE queue: k-side consts, then the eight x strips.
            nc.sync.dma_start(wk_sb[:], wk_d)
            nc.sync.dma_start(bk_sb[:], bk_d)
            nc.sync.dma_start(wv_sb[:], wv_d)
            nc.sync.dma_start(bvb_sb[:], bvb_d)
            for st in range(NSTRIP):
                nc.sync.dma_start(xt_t[st][:], xT_d[st])
            for tt in range(NT):
                nc.gpsimd.memset(v_t[tt][:, 128:129], 1.0)

            # PE warmup: throwaway matmuls on scratch keep the PE busy from
            # the end of the framework preamble so the HAM clock gate opens
            # (2.4 GHz) before the first x strip lands.
            with tc.tile_pool(name="warm", bufs=2, space="PSUM") as wpp:
                wps = [wpp.tile([P, 512], f32, tag=f"w{i}", name=f"warm_{i}")
                       for i in range(2)]
                for i in range(NWARM):
                    nc.tensor.matmul(wps[i % 2], warm_sb[:, 0:128], warm_sb[:],
                                     start=True, stop=True)

            with tc.tile_pool(name="ps_o", bufs=2, space="PSUM") as po_pool, \
                 tc.tile_pool(name="sb_w", bufs=6) as wpool, \
                 tc.tile_pool(name="sb_f", bufs=2) as fpool:

                o_sbg = [fpool.tile([P, 4, P], f16, tag=f"og{g}",
                                    name=f"osb_{g}") for g in range(4)]

                def finalize(g, ci, o_acc):
                    rec = fpool.tile([P, 1], f32, tag="rec",
                                     name=f"rec_{g}_{ci}")
                    nc.vector.reciprocal(rec, o_acc[:, 128:129])
                    nc.vector.tensor_scalar_mul(o_sbg[g][:, ci],
                                                o_acc[:, 0:128], rec)
                    if g == 3:    # tail group: per-slot DMAs drain earlier
                        nc.sync.dma_start(out_d[g, :, ci], o_sbg[g][:, ci])
                    elif ci == 0:  # slots[0] has the largest KC: finishes last
                        nc.sync.dma_start(out_d[g], o_sbg[g])

                def build_masks():
                    for blk in range(NS * 2):
                        nc.vector.tensor_scalar(
                            msk_sb[:, ts(blk, P)], tril_sb,
                            ab_sb[:, 2 * blk:2 * blk + 1],
                            ab_sb[:, 2 * blk + 1:2 * blk + 2],
                            mybir.AluOpType.mult, mybir.AluOpType.add)

                def boundary_masks(p_ap_fn, u, slots, w):
                    for ci, s in enumerate(slots[:w]):
                        if u >= KC[s] - 2:
                            i = u - (KC[s] - 2)
                            nc.vector.tensor_mul(
                                p_ap_fn(ci), p_ap_fn(ci),
                                msk_sb[:, ds((2 * s + i) * P, P)])

                def proj_q(pp, st):
                    ps = pp.tile([P, 512], f32, tag="proj", name=f"psq_{st}")
                    for o in range(DO):
                        nc.tensor.matmul(ps, wq_sb[:, o], xq_t[st][:, o],
                                         start=(o == 0), stop=(o == DO - 1))
                    nc.vector.tensor_scalar_add(qt_t[st][:], ps, bq_sb)

                def proj_kv(pp, st):
                    ps = pp.tile([P, 512], f32, tag="proj", name=f"psk_{st}")
                    for o in range(DO):
                        nc.tensor.matmul(ps, wk_sb[:, o], xt_t[st][:, o],
                                         start=(o == 0), stop=(o == DO - 1))
                    nc.vector.tensor_scalar_add(kt_t[st][:], ps, bk_sb)
                    for j in range(4):
                        tt = 4 * st + j
                        ps_v = pp.tile([P, P], f32, tag="proj", name=f"psv_{tt}")
                        for o in range(DO):
                            nc.tensor.matmul(ps_v, xt_t[st][:, o, ts(j, P)],
                                             wv_sb[:, o],
                                             start=(o == 0), stop=(o == DO - 1))
                        nc.vector.tensor_add(v_t[tt][:, 0:128], ps_v, bvb_sb)

                # ---- phase 1: projections with group 0 interleaved ----
                slots0 = [0, 1, 2, 3]
                ob0 = {i: po_pool.tile([P, 2, 129], f32, tag="oacc",
                                       name=f"o_bank_0_{i}") for i in range(2)}
                o_ps0 = {s: ob0[ci // 2][:, ci % 2]
                         for ci, s in enumerate(slots0)}
                first0 = {0, 2}
                with tc.tile_pool(name="pproj", bufs=2, space="PSUM") as pp, \
                     tc.tile_pool(name="ps_s0", bufs=2, space="PSUM") as ps0:
                    for st in range(NSTRIP):
                        if st < NQSTRIP:
                            proj_q(pp, st)
                        proj_kv(pp, st)
                        if st == 4:
                            build_masks()
                        for up in (2 * st, 2 * st + 1):   # g0 pairs
                            u0 = 2 * up
                            w = sum(1 for s in slots0 if KC[s] > u0)
                            s_sp = ps0.tile([P, 2, 512], f32, tag="s0",
                                            name=f"s0_{u0}")
                            for j in range(2):
                                u = u0 + j
                                nc.tensor.matmul(s_sp[:, j, 0:w * P],
                                                 kt_t[st][:, ts(u % 4, P)],
                                                 qt_t[0][:, 0:w * P],
                                                 start=True, stop=True)
                            p_sb = wpool.tile([P, 2, 512], f16, tag="ptile0",
                                              name=f"p0_{u0}")
                            nc.scalar.activation(p_sb[:, :, 0:w * P],
                                                 s_sp[:, :, 0:w * P], Exp)
                            for j in range(2):
                                boundary_masks(
                                    lambda ci, j=j: p_sb[:, j, ts(ci, P)],
                                    u0 + j, slots0, w)
                            for j in range(2):
                                u = u0 + j
                                for ci, s in enumerate(slots0[:w]):
                                    nc.tensor.matmul(
                                        o_ps0[s], p_sb[:, j, ts(ci, P)],
                                        v_t[u][:, 0:129],
                                        start=(u == 0 and s in first0),
                                        stop=(u == KC[s] - 1),
                                        skip_group_check=True)
                                    if u == KC[s] - 1:
                                        finalize(0, ci, o_ps0[s])

                # ---- phase 2: attention, three k-tiles per exp ----
                with tc.tile_pool(name="ps_s", bufs=2, space="PSUM") as ps_pool:
                    for g in range(1, 4):
                        slots = list(range(4 * g, 4 * g + 4))
                        o_bank = {i: po_pool.tile([P, 2, 129], f32, tag="oacc",
                                                  name=f"o_bank_{g}_{i}")
                                  for i in range(2)}
                        o_ps = {s: o_bank[ci // 2][:, ci % 2]
                                for ci, s in enumerate(slots)}
                        first_in_bank = {slots[0], slots[2]}
                        c0 = KC[slots[0]]
                        for u0 in range(0, c0, 3):     # k-tile triples
                            span = min(3, c0 - u0)
                            w0 = sum(1 for s in slots if KC[s] > u0)
                            s_ps = ps_pool.tile([P, 3, 512], f32, tag="sacc",
                                                name=f"s_{g}_{u0}")
                            for j in range(span):
                                u = u0 + j
                                wj = sum(1 for s in slots if KC[s] > u)
                                nc.tensor.matmul(s_ps[:, j, 0:wj * P],
                                                 kt_t[u // 4][:, ts(u % 4, P)],
                                                 qt_t[g][:, 0:wj * P],
                                                 start=True, stop=True)
                            p_sb = wpool.tile([P, 3, 512], f16, tag="ptile",
                                              name=f"p_{g}_{u0}")
                            nc.scalar.activation(p_sb[:, 0:span, 0:w0 * P],
                                                 s_ps[:, 0:span, 0:w0 * P],
                                                 Exp)
                            for j in range(span):
                                boundary_masks(
                                    lambda ci, j=j: p_sb[:, j, ts(ci, P)],
                                    u0 + j, slots,
                                    sum(1 for s in slots if KC[s] > u0 + j))
                            for j in range(span):
                                u = u0 + j
                                wj = sum(1 for s in slots if KC[s] > u)
                                for ci, s in enumerate(slots[:wj]):
                                    nc.tensor.matmul(
                                        o_ps[s], p_sb[:, j, ts(ci, P)],
                                        v_t[u][:, 0:129],
                                        start=(u == 0 and s in first_in_bank),
                                        stop=(u == KC[s] - 1),
                                        skip_group_check=True)
                                    if u == KC[s] - 1:
                                        finalize(g, ci, o_ps[s])

    nc.compile()
    return nc


def _prep_core(core, x, wq, bq, wk, bk, wv, bv):
    b, ct = core // 2, core % 2
    qtiles = _slot_qtile(ct)
    scale = np.float32(1.0 / np.sqrt(H))

    def dchunk(a):  # [D, N] -> [P, DO, N] with d = o*P + p
        return np.ascontiguousarray(
            a.reshape(DO, P, -1).transpose(1, 0, 2)).astype(_f16)

    def stripmajor(a, nstrip):  # [P, DO, N] -> [nstrip, P, DO, 512]
        return np.ascontiguousarray(
            a.reshape(P, DO, nstrip, 512).transpose(2, 0, 1, 3))

    xT = x[b].T.astype(np.float32)                      # [D, T]
    qrows = np.concatenate([np.arange(j * P, (j + 1) * P) for j in qtiles])
    xqT = np.ascontiguousarray(xT[:, qrows])            # [D, TQ]

    # per-block mask = tril*b + a: (a,b) = (1,0) ones / (0,1) triu / (0,0)
    # zeros — built on-device from one triu tile to keep DMA input small.
    ab = np.zeros((P, NS * 2 * 2), dtype=np.float32)
    for s in range(NS):
        j = qtiles[s]
        for i in range(2):
            u = KC[s] - 2 + i
            blk = 2 * s + i
            if u < j:
                ab[:, 2 * blk + 1] = 1.0
            elif u == j:
                ab[:, 2 * blk] = 1.0

    return {
        "xT": stripmajor(dchunk(xT), NSTRIP),
        "xqT": stripmajor(dchunk(xqT), NQSTRIP),
        "wq": dchunk(wq * scale),
        "wk": dchunk(wk),
        "wv": dchunk(wv),
        "bq": (bq * scale).astype(np.float32).reshape(P, 1),
        "bk": bk.astype(np.float32).reshape(P, 1),
        "bvb": np.tile(bv.astype(np.float32), (P, 1)),
        "tril": np.triu(np.ones((P, P), dtype=_f16)),
        "ab": ab,
    }


def _fallback(x, mask, wq, bq, wk, bk, wv, bv):
    """Exact numpy path for inputs the specialized kernel doesn't cover."""
    out = np.empty((x.shape[0], x.shape[1], wq.shape[1]), dtype=np.float32)
    scale = np.float32(1.0 / np.sqrt(wq.shape[1]))
    for b in range(x.shape[0]):
        q = x[b] @ wq + bq
        k = x[b] @ wk + bk
        v = x[b] @ wv + bv
        s = (q @ k.T) * scale
        s = np.where(mask == 0, np.float32(-1e30), s)
        s -= s.max(axis=-1, keepdims=True)
        p = np.exp(s)
        p /= p.sum(axis=-1, keepdims=True)
        out[b] = p @ v
    return out


def kernel(**inputs):
    x = np.asarray(inputs["x"], dtype=np.float32)
    mask = np.asarray(inputs["mask"])
    wq = np.asarray(inputs["wq"], dtype=np.float32)
    bq = np.asarray(inputs["bq"], dtype=np.float32)
    wk = np.asarray(inputs["wk"], dtype=np.float32)
    bk = np.asarray(inputs["bk"], dtype=np.float32)
    wv = np.asarray(inputs["wv"], dtype=np.float32)
    bv = np.asarray(inputs["bv"], dtype=np.float32)

    causal = (x.shape == (B, T, D) and wq.shape == (D, H)
              and np.array_equal(mask, np.tril(np.ones((T, T), mask.dtype))))
    if not causal:
        return _fallback(x, mask, wq, bq, wk, bk, wv, bv)

    if "nc" not in _CACHE:
        _CACHE["nc"] = _build_program()
    nc = _CACHE["nc"]

    from concourse import bass_utils
    in_maps = [_prep_core(c, x, wq, bq, wk, bk, wv, bv) for c in range(8)]
    res = bass_utils.run_bass_kernel_spmd(nc, in_maps, core_ids=list(range(8)),
                                          **_CACHE.get("run_kwargs", {}))
    _CACHE["last_result"] = res

    out = np.empty((B, T, H), dtype=np.float32)
    for c in range(8):
        b, ct = c // 2, c % 2
        qtiles = _slot_qtile(ct)
        oc = res.results[c]["out"].astype(np.float32)   # [NQSTRIP, P, 4, P]
        for s, j in enumerate(qtiles):
            out[b, j * P:(j + 1) * P, :] = oc[s // 4, :, s % 4]
    return out
